# revision 30
# baseline (speedup 1.0000x reference)
"""GQA kernel for Trainium2, sharded over 8 NeuronCores.

Problem: B=2, S=2048, H=2048, NH=16 q-heads, KVH=4 kv-heads, D=128.
Sharding: core c -> (batch b = c//4, kv-head k = c%4). Each core computes the
full attention for its 4 query heads + its kv head on its batch, plus the
row-parallel partial of the output projection. Host sums the 4 partials per
batch and adds the output bias.

Final design (single fused pass per 512-token q-chunk):
  startup: PE warm-up dummy matmuls during the initial DMA wait (HAM
  unthrottles 1.2->2.4GHz before real work); exp activation table preloaded;
  chunk-0 weights/hidden DMA'd in progressively sized k-tile groups so the
  first matmuls start after ~330KB while the rest streams.
  for c in 0..3:
    prefetch h(c+1); QKV projection + RoPE for chunk c (rows k,v,q0..q3;
    k/v rows accumulate in the attn@V pool, free at the chunk boundary);
    the previous chunk's deferred hp1 normalization lands behind the k-row
    matmuls so its denominator matmuls never stall the tensor queue;
    transpose v tiles of chunk c;
    attention for chunk c in two head-pair sweeps with oproj(c-1) micro-ops
    (one [128,512] output-column block: 4 accumulated matmuls + copy)
    interleaved between kv-tile iterations to fill the exp-wait bubbles
    (unused micro-ops carry over to later chunks):
      per kv tile j: 2 score matmuls (kT[j] stationary), ONE exp over the
      [128, 2, w] head-pair mega-tile (causally trimmed width w), triangular
      mask on the 128-wide diagonal block only, denominator accumulated in
      bf16 on DVE as independent even/odd chains, attn@V accumulated in PSUM
      (lagging one j so the matmul never waits on exp);
      per head: two accumulated ones-matmuls merge the chains ->
      reciprocal_approx_fast -> gpsimd partition_broadcast -> normalize into
      xT (bf16); hp0's norm is emitted behind hp1's sweep, hp1's is deferred
      into the next chunk's QKV.
  trailing oproj(3), with the final norm broadcast done via a tensor-engine
  matmul (avoids a gpsimd pipeline drain on the critical tail).
Output partials are written as bf16; host upcasts, sums, and adds bo.

Measured on TRN2 (8 cores): 248.7us vs 437.6us for the phase-separated
baseline (tensor-engine active ~227us of a ~209us matmul-issue floor).
"""

import numpy as np
import ml_dtypes

import concourse.bass as bass
import concourse.mybir as mybir
import concourse.tile as tile
from concourse import bacc

BF16 = ml_dtypes.bfloat16
F32 = mybir.dt.float32
BF = mybir.dt.bfloat16

B, S, H = 2, 2048, 2048
NH, KVH, D = 16, 4, 128
G = NH // KVH  # q heads per kv head / per core
N_CORES = 8
SCALE = 1.0 / float(np.sqrt(D))

SQ = 512              # q-chunk width
NQC = S // SQ         # 4 q chunks
NKT = S // 128        # 16 kv tiles / token tiles
NHT = H // 128        # 16 hidden k-tiles
ROWS = G + 2          # 6 projection row-blocks: 4 q heads, k, v
EXPF = mybir.ActivationFunctionType.Exp
IDF = mybir.ActivationFunctionType.Identity


def build_nc(num_devices: int = N_CORES) -> bass.Bass:
    nc = bacc.Bacc("TRN2", num_devices=num_devices)

    # packed layouts: [partition, tile, cols] so one DMA moves many tiles
    hTd = nc.dram_tensor("hTd", [128, NHT, S], BF, kind="ExternalInput").ap()
    wqd = nc.dram_tensor("wqd", [128, NHT, ROWS * 128], BF,
                         kind="ExternalInput").ap()
    bqkv = nc.dram_tensor("bqkv", [128, ROWS], F32, kind="ExternalInput").ap()
    cosT = nc.dram_tensor("cosT", [128, S], BF, kind="ExternalInput").ap()
    sinT = nc.dram_tensor("sinT", [128, S], BF, kind="ExternalInput").ap()
    rotT = nc.dram_tensor("rotT", [128, 128], BF, kind="ExternalInput").ap()
    masks2 = nc.dram_tensor("masks2", [128, 256], BF, kind="ExternalInput").ap()
    wod = nc.dram_tensor("wod", [128, G, H], BF, kind="ExternalInput").ap()
    id128 = nc.dram_tensor("id128", [128, 128], BF, kind="ExternalInput").ap()
    out = nc.dram_tensor("out", [S, H], BF, kind="ExternalOutput").ap()

    with tile.TileContext(nc) as tc:
        with (
            tc.tile_pool(name="consts", bufs=1) as consts,
            tc.tile_pool(name="persist", bufs=1) as persist,
            tc.tile_pool(name="hbuf", bufs=2) as hbuf,
            tc.tile_pool(name="work", bufs=4) as work,
            tc.tile_pool(name="work2", bufs=2) as work2,
            tc.tile_pool(name="obuf", bufs=4) as obuf,
            tc.tile_pool(name="psQ", bufs=2, space="PSUM") as psQ,
            tc.tile_pool(name="psS", bufs=2, space="PSUM") as psS,
            tc.tile_pool(name="psAV", bufs=2, space="PSUM") as psAV,
        ):
            ones_f = consts.tile([128, 1], BF, tag="ones_f", name="ones_f")
            nc.vector.memset(ones_f, 1.0)
            ones_rf = consts.tile([1, 128], F32, tag="ones_rf", name="ones_rf")
            nc.vector.memset(ones_rf, 1.0)
            # preload the exp activation table while DMA streams
            warm_in = consts.tile([128, 1], F32, tag="warm_in", name="warm_in")
            nc.vector.memset(warm_in, 0.0)
            warm_out = consts.tile([128, 1], BF, tag="warm_out", name="warm_out")
            nc.scalar.activation(warm_out, warm_in, EXPF)
            # PE warm-up: dummy matmuls on memset data during the initial DMA
            # wait so HAM unthrottles (1.2 -> 2.4 GHz) before real work
            dummy_w = consts.tile([128, SQ], BF, tag="dummy", name="dummy")
            nc.vector.memset(dummy_w, 0.0)
            for wi in range(10):
                dps = psQ.tile([128, SQ], F32, tag="qkv", name="warmmm")
                nc.tensor.matmul(dps, dummy_w[:, 0:128], dummy_w,
                                 start=True, stop=True)

            # weights + chunk-0 hidden: progressively sized k-tile groups so
            # the first matmuls start after ~330KB while the rest streams
            GRP = [(0, 1), (1, 2), (2, 3), (3, 5), (5, 8), (8, 12), (12, 16)]
            wq_g, h_g = [], []
            h_q = [[None] * 4 for _ in range(NQC)]
            for gi, (lo, hi) in enumerate(GRP):
                wt = persist.tile([128, hi - lo, ROWS * 128], BF,
                                  tag=f"wq{gi}", name=f"wq{gi}")
                nc.sync.dma_start(out=wt, in_=wqd[:, lo:hi, :])
                wq_g.append(wt)
                ht = persist.tile([128, hi - lo, SQ], BF, tag=f"hg{gi}",
                                  name=f"h0_{gi}")
                nc.sync.dma_start(out=ht, in_=hTd[:, lo:hi, 0:SQ])
                h_g.append(ht)
            # small constants: not needed until ~25us in, so their DMA
            # triggers go after the critical weight/hidden ones
            bias_sb = consts.tile([128, ROWS], F32, tag="bias", name="bias")
            nc.sync.dma_start(out=bias_sb, in_=bqkv)
            rt_sb = consts.tile([128, 128], BF, tag="rt", name="rt")
            nc.sync.dma_start(out=rt_sb, in_=rotT)
            mask_sb = consts.tile([128, 2, 128], BF, tag="mask", name="mask")
            nc.sync.dma_start(out=mask_sb, in_=masks2)
            id_sb = consts.tile([128, 128], BF, tag="id", name="id")
            nc.sync.dma_start(out=id_sb, in_=id128)
            cos_sb = persist.tile([128, S], BF, tag="cos", name="cos")
            nc.sync.dma_start(out=cos_sb, in_=cosT)
            sin_sb = persist.tile([128, S], BF, tag="sin", name="sin")
            nc.sync.dma_start(out=sin_sb, in_=sinT)
            wo_sb = persist.tile([128, G, H], BF, tag="wo", name="wo")
            nc.sync.dma_start(out=wo_sb, in_=wod)

            def _grp(kt):
                for gi, (lo, hi) in enumerate(GRP):
                    if kt < hi:
                        return gi, kt - lo
                raise AssertionError

            def wq_ap(kt, m):
                gi, o = _grp(kt)
                return wq_g[gi][:, o, m * 128:(m + 1) * 128]

            def h_ap(c, kt):
                if c == 0:
                    gi, o = _grp(kt)
                    return h_g[gi][:, o, :]
                return h_q[c][kt // 4][:, kt % 4, :]

            # persistent activations (bf16)
            qk_sb = [persist.tile([128, S], BF, tag=f"qk{m}", name=f"qk{m}")
                     for m in range(G + 1)]  # 0..3 q heads, 4 = k
            vT_sb = persist.tile([128, S], BF, tag="vT", name="vT")
            v_sb = [persist.tile([128, 128], BF, tag=f"v{j}", name=f"v{j}")
                    for j in range(NKT)]
            xT_sb = [persist.tile([128, S], BF, tag=f"xT{h}", name=f"xT{h}")
                     for h in range(G)]
            kT = qk_sb[G]

            # rows: m 0..3 -> q head m (RoPE), 4 -> k (RoPE), 5 -> v (plain)
            def row_bias(m, ps, cs):
                """PSUM->SBUF copy with bias; returns rope tmp or None."""
                if m == ROWS - 1:
                    nc.scalar.activation(vT_sb[:, cs], ps, IDF,
                                         bias=bias_sb[:, m:m + 1])
                    return None
                tmp = work.tile([128, SQ], BF, tag="tmp", name="tmp")
                nc.scalar.activation(tmp, ps, IDF, bias=bias_sb[:, m:m + 1])
                return tmp

            def row_rope(m, tmp, cs):
                rp = psAV.tile([128, SQ], F32, tag="av", name="rp")
                nc.tensor.matmul(rp, rt_sb, tmp, start=True, stop=True)
                rot = work.tile([128, SQ], BF, tag="rot", name="rot")
                nc.vector.tensor_mul(rot, rp, sin_sb[:, cs])
                tcos = work.tile([128, SQ], BF, tag="tcos", name="tcos")
                nc.vector.tensor_mul(tcos, tmp, cos_sb[:, cs])
                nc.vector.tensor_add(qk_sb[min(m, G)][:, cs], rot, tcos)

            def oproj_fill_ops(c, fine_dma=False):
                """One micro-op per (token tile, output column block): alloc a
                PSUM tile, 4 accumulated matmuls over heads, copy into the
                per-tile output staging buffer, DMA the row block when done.
                Each closure takes the PSUM pool to use (the one that is free
                during the sweep it is interleaved into)."""
                osbs = {}
                ops = []
                for ti, t in enumerate(range(4 * c, 4 * c + 4)):
                    for n in range(G):
                        def op_fn(pool, t=t, n=n, fine_dma=fine_dma):
                            if t not in osbs:
                                osbs[t] = obuf.tile([128, H], BF, tag="osb",
                                                    name="osb")
                            osb = osbs[t]
                            op = pool.tile([128, SQ], F32,
                                           tag="qkv" if pool is psQ else "av",
                                           name="op")
                            ts_ = slice(t * 128, (t + 1) * 128)
                            for g in range(G):
                                nc.tensor.matmul(
                                    op, xT_sb[g][:, ts_],
                                    wo_sb[:, g, n * SQ:(n + 1) * SQ],
                                    start=(g == 0), stop=(g == G - 1))
                            dst = osb[:, n * SQ:(n + 1) * SQ]
                            if n % 2 == 0:
                                nc.scalar.copy(dst, op)
                            else:
                                nc.vector.tensor_copy(dst, op)
                            if fine_dma:
                                nc.sync.dma_start(
                                    out=out[t * 128:(t + 1) * 128,
                                            n * SQ:(n + 1) * SQ],
                                    in_=osb[:, n * SQ:(n + 1) * SQ])
                            elif n == G - 1:
                                nc.sync.dma_start(
                                    out=out[t * 128:(t + 1) * 128, :], in_=osb)
                        ops.append(op_fn)
                return ops

            def attn_chunk(c, fill, between=None):
                """Returns the hp1 normalization closure for deferred
                emission (behind the next chunk's first QKV row) so the
                denominator matmuls never stall the tensor queue. `between`
                is emitted after the hp0 sweep (before hp1's)."""
                cs = slice(c * SQ, (c + 1) * SQ)
                njt = 4 * c + 4
                split = njt >= 8  # even/odd denominator chains (j=0,1 full)
                norms = []
                for hp in range(2):
                    if hp == 1 and between is not None:
                        between()
                    h0, h1 = 2 * hp, 2 * hp + 1
                    if hp == 0:
                        av0 = psAV.tile([128, SQ], F32, tag="av", name="av0")
                        av1 = psAV.tile([128, SQ], F32, tag="av", name="av1")
                    else:
                        av0 = psQ.tile([128, SQ], F32, tag="qkv", name="av0b")
                        av1 = psQ.tile([128, SQ], F32, tag="qkv", name="av1b")
                    daccs = [work2.tile([128, 2, SQ], BF, tag=f"dacc{p}",
                                        name=f"dacc{p}")
                             for p in range(2 if split else 1)]
                    pend = None  # (j, ex, off) awaiting its attn@V matmuls
                    for j in range(njt):
                        i = j - 4 * c
                        off = 128 * i if i > 0 else 0
                        sc = psS.tile([128, 2, SQ], F32, tag="sc", name="sc")
                        for hs, h in ((0, h0), (1, h1)):
                            nc.tensor.matmul(
                                sc[:, hs, off:],
                                kT[:, j * 128:(j + 1) * 128],
                                qk_sb[h][:, c * SQ + off:(c + 1) * SQ],
                                start=True, stop=True,
                            )
                        if pend is not None:
                            pj, pex, poff = pend
                            nc.tensor.matmul(av0[:, poff:], v_sb[pj],
                                             pex[:, 0, poff:],
                                             start=(pj == 0), stop=False)
                            nc.tensor.matmul(av1[:, poff:], v_sb[pj],
                                             pex[:, 1, poff:],
                                             start=(pj == 0), stop=False)
                        remaining = (njt - j) + (njt - 1 if hp == 0 else 0)
                        if j >= 1 and fill and (j % 2 == 1
                                                or len(fill) >= remaining):
                            # psQ is free during hp0 (held by hp1's attn@V
                            # accumulators later); psAV frees once hp0's
                            # normalization has read av0/av1. Spread the ops
                            # evenly so neither sweep runs exp-gated dry.
                            fill.pop(0)(psQ if hp == 0 else psAV)
                        ex = work.tile([128, 2, SQ], BF, tag="ex", name="ex")
                        nc.scalar.activation(ex[:, :, off:], sc[:, :, off:],
                                             EXPF, scale=SCALE)
                        if i >= 0:
                            nc.vector.tensor_mul(ex[:, :, off:off + 128],
                                                 ex[:, :, off:off + 128],
                                                 mask_sb)
                        dacc = daccs[j % 2] if split else daccs[0]
                        if j < (2 if split else 1):
                            nc.vector.tensor_copy(dacc, ex)
                        else:
                            nc.vector.tensor_add(dacc[:, :, off:],
                                                 dacc[:, :, off:],
                                                 ex[:, :, off:])
                        pend = (j, ex, off)
                        if hp == 1 and j == 1:
                            norms[0]()  # hp0 norm behind hp1's first matmuls
                    pj, pex, poff = pend
                    nc.tensor.matmul(av0[:, poff:], v_sb[pj], pex[:, 0, poff:],
                                     start=(pj == 0), stop=True)
                    nc.tensor.matmul(av1[:, poff:], v_sb[pj], pex[:, 1, poff:],
                                     start=(pj == 0), stop=True)

                    def norm(hp=hp, av0=av0, av1=av1, daccs=daccs,
                             mm_bcast=False):
                        if len(daccs) == 2:
                            dm = work2.tile([128, 2, SQ], BF, tag="daccm",
                                            name="daccm")
                            nc.vector.tensor_add(dm, daccs[0], daccs[1])
                        else:
                            dm = daccs[0]
                        for hs, av in ((0, av0), (1, av1)):
                            h = 2 * hp + hs
                            dn = psS.tile([1, SQ], F32, tag="sc", name="dn")
                            nc.tensor.matmul(dn, ones_f, dm[:, hs, :],
                                             start=True, stop=True)
                            rd = work2.tile([1, SQ], F32, tag="rd", name="rd")
                            nc.vector.reciprocal_approx_fast(rd, dn)
                            rdb = work2.tile([128, SQ], F32, tag="rdb",
                                             name="rdb")
                            if mm_bcast:
                                # tensor-engine broadcast: avoids the gpsimd
                                # pipeline drain on the kernel's critical tail
                                bc = psAV.tile([128, SQ], F32, tag="av",
                                               name="bc")
                                nc.tensor.matmul(bc, ones_rf, rd,
                                                 start=True, stop=True)
                                nc.scalar.copy(rdb, bc)
                            else:
                                nc.gpsimd.partition_broadcast(rdb, rd)
                            nc.vector.tensor_mul(xT_sb[h][:, cs], av, rdb)
                    norms.append(norm)
                return norms[1]

            ROW_ORDER = (G, ROWS - 1, 0, 1, 2, 3)  # k, v, q0..q3
            pending_norm = None
            fill = []  # oproj micro-ops carried across chunks as filler
            for c in range(NQC):
                cs = slice(c * SQ, (c + 1) * SQ)
                # prefetch next chunk's hidden tiles
                if c + 1 < NQC:
                    for q in range(4):
                        ht = hbuf.tile([128, 4, SQ], BF, tag=f"h{q}",
                                       name=f"h{c + 1}_{q}")
                        nc.sync.dma_start(
                            out=ht,
                            in_=hTd[:, 4 * q:4 * q + 4,
                                    (c + 1) * SQ:(c + 2) * SQ])
                        h_q[c + 1][q] = ht
                # ---- QKV projection + RoPE ----
                if c == 0:
                    # k-tile-outer so compute starts as DMA streams in;
                    # 6 concurrent accumulators spread over all three pools
                    pools = {0: psQ, 1: psQ, 2: psS, 3: psS, 4: psAV, 5: psAV}
                    tags = {0: "qkv", 1: "qkv", 2: "sc", 3: "sc",
                            4: "av", 5: "av"}
                    accs = {m: pools[m].tile([128, SQ], F32, tag=tags[m],
                                             name=f"acc{m}")
                            for m in range(ROWS)}
                    for kt in range(NHT):
                        for m in range(ROWS):
                            nc.tensor.matmul(
                                accs[m], wq_ap(kt, m), h_ap(0, kt),
                                start=(kt == 0), stop=(kt == NHT - 1),
                            )
                    tmps = {m: row_bias(m, accs[m], cs) for m in ROW_ORDER}
                    for m in ROW_ORDER:
                        if tmps[m] is not None:
                            row_rope(m, tmps[m], cs)
                else:
                    prev = None  # stagger rope behind next row's matmuls
                    rows = ROW_ORDER
                    if c == 1:
                        rows = ROW_ORDER[1:]  # k was hoisted into attn(0)
                        prev = (G, ktmp_cell[0])
                    for m in rows:
                        # k and v rows accumulate in the attn@V pool, which
                        # is free at the chunk boundary; q rows wait for the
                        # deferred norm to release the psQ slots
                        pool, tag = (psAV, "av") if m >= G else (psQ, "qkv")
                        ps = pool.tile([128, SQ], F32, tag=tag, name="mm")
                        for kt in range(NHT):
                            nc.tensor.matmul(
                                ps, wq_ap(kt, m), h_ap(c, kt),
                                start=(kt == 0), stop=(kt == NHT - 1),
                            )
                        if pending_norm is not None:
                            # prev chunk's hp1 norm: its denominator matmuls
                            # land behind this k-row so they never wait
                            pending_norm()
                            pending_norm = None
                        if prev is not None:
                            row_rope(prev[0], prev[1], cs)
                            prev = None
                        tmp = row_bias(m, ps, cs)
                        if tmp is not None:
                            prev = (m, tmp)
                    if prev is not None:
                        row_rope(prev[0], prev[1], cs)
                # ---- transpose this chunk's v tiles ----
                for j in range(4 * c, 4 * c + 4):
                    tp = psS.tile([128, 128], BF, tag="sc", name="tp")
                    nc.tensor.transpose(tp, vT_sb[:, j * 128:(j + 1) * 128],
                                        id_sb)
                    nc.scalar.copy(v_sb[j], tp)
                # ---- attention with oproj(c-1) interleaved ----
                between = None
                if c == 0:
                    # hoist chunk-1's k-row projection into attn(0)'s hp0
                    # sweep (psQ is free there); its PSUM->SBUF bias copy is
                    # emitted between the sweeps so hp1's attn@V accumulators
                    # get the slot back, and its RoPE runs in chunk 1
                    kacc_cell, ktmp_cell = [], []

                    def k_fill(pool, half=0):
                        if not kacc_cell:
                            kacc_cell.append(psQ.tile([128, SQ], F32,
                                                      tag="qkv", name="kmm"))
                        ka = kacc_cell[0]
                        for kt in range(8 * half, 8 * half + 8):
                            nc.tensor.matmul(ka, wq_ap(kt, G), h_ap(1, kt),
                                             start=(kt == 0),
                                             stop=(kt == NHT - 1))

                    fill = [lambda pool: k_fill(pool, 0),
                            lambda pool: k_fill(pool, 1)]

                    def between():
                        ktmp_cell.append(row_bias(G, kacc_cell[0],
                                                  slice(SQ, 2 * SQ)))
                else:
                    fill.extend(oproj_fill_ops(c - 1))
                pending_norm = attn_chunk(c, fill, between)
            pending_norm(mm_bcast=True)
            for fi, fn in enumerate(oproj_fill_ops(NQC - 1, fine_dma=True)):
                fn(psQ if fi % 2 else psAV)
    nc.compile()
    return nc


def make_in_maps(hidden_states, cos, sin, Wq, bq, Wk, bk, Wv, bv, Wo, bo):
    """Host-side shard/pack. Returns list of 8 input dicts."""
    f32 = np.float32
    cosT = np.ascontiguousarray(np.asarray(cos).T).astype(BF16)
    sinT = np.ascontiguousarray(np.asarray(sin).T).astype(BF16)
    R = np.zeros((128, 128), f32)
    for d in range(64):
        R[d, d + 64] = -1.0
        R[d + 64, d] = 1.0
    rotT = np.ascontiguousarray(R.T).astype(BF16)
    # triangular mask for the diagonal 128-block, duplicated per head-pair
    p = np.arange(128)[:, None]
    q = np.arange(128)[None, :]
    tri = (q >= p).astype(BF16)
    masks2 = np.concatenate([tri, tri], axis=1)  # [128, 256]
    id128 = np.eye(128, dtype=BF16)

    in_maps = []
    for core in range(N_CORES):
        b, k = core // 4, core % 4
        hT = np.ascontiguousarray(np.asarray(hidden_states[b]).T)  # [H, S]
        hTd = np.ascontiguousarray(
            hT.reshape(NHT, 128, S).transpose(1, 0, 2)).astype(BF16)
        wq = Wq[512 * k:512 * (k + 1)]            # [512, H]
        wk = Wk[128 * k:128 * (k + 1)]            # [128, H]
        wv = Wv[128 * k:128 * (k + 1)]
        wqkvT = np.ascontiguousarray(
            np.concatenate([wq, wk, wv], axis=0).T)  # [H, 768]
        wqd = np.ascontiguousarray(
            wqkvT.reshape(NHT, 128, ROWS * 128).transpose(1, 0, 2)
        ).astype(BF16)                             # [128, 16, 768]
        bqkv = np.concatenate(
            [bq[512 * k:512 * (k + 1)], bk[128 * k:128 * (k + 1)],
             bv[128 * k:128 * (k + 1)]]
        ).astype(f32).reshape(ROWS, 128).T.copy()  # [128, ROWS]
        woT = np.ascontiguousarray(Wo[:, 512 * k:512 * (k + 1)].T)  # [512, H]
        wod = np.ascontiguousarray(
            woT.reshape(G, 128, H).transpose(1, 0, 2)).astype(BF16)
        in_maps.append({
            "hTd": hTd, "wqd": wqd, "bqkv": bqkv,
            "cosT": cosT, "sinT": sinT, "masks2": masks2, "rotT": rotT,
            "wod": wod, "id128": id128,
        })
    return in_maps


_NC = None


def kernel(**inputs) -> np.ndarray:
    global _NC
    from concourse.bass_utils import run_bass_kernel_spmd

    if _NC is None:
        _NC = build_nc()
    in_maps = make_in_maps(**inputs)
    res = run_bass_kernel_spmd(_NC, in_maps, core_ids=list(range(N_CORES)))
    out = np.zeros((B, S, H), np.float32)
    for core in range(N_CORES):
        out[core // 4] += np.asarray(res.results[core]["out"], np.float32)
    out += np.asarray(inputs["bo"], np.float32)
    return out


# revision 32
# speedup vs baseline: 1.0001x; 1.0001x over previous
"""GQA kernel for Trainium2, sharded over 8 NeuronCores.

Problem: B=2, S=2048, H=2048, NH=16 q-heads, KVH=4 kv-heads, D=128.
Sharding: core c -> (batch b = c//4, kv-head k = c%4). Each core computes the
full attention for its 4 query heads + its kv head on its batch, plus the
row-parallel partial of the output projection. Host sums the 4 partials per
batch and adds the output bias.

Final design (single fused pass per 512-token q-chunk):
  startup: PE warm-up dummy matmuls during the initial DMA wait (HAM
  unthrottles 1.2->2.4GHz before real work); exp activation table preloaded;
  chunk-0 weights/hidden DMA'd in progressively sized k-tile groups so the
  first matmuls start after ~330KB while the rest streams.
  for c in 0..3:
    prefetch h(c+1); QKV projection + RoPE for chunk c (rows k,v,q0..q3;
    k/v rows accumulate in the attn@V pool, free at the chunk boundary);
    the previous chunk's deferred hp1 normalization lands behind the k-row
    matmuls so its denominator matmuls never stall the tensor queue;
    transpose v tiles of chunk c;
    attention for chunk c in two head-pair sweeps with oproj(c-1) micro-ops
    (one [128,512] output-column block: 4 accumulated matmuls + copy)
    interleaved between kv-tile iterations to fill the exp-wait bubbles
    (unused micro-ops carry over to later chunks):
      per kv tile j: 2 score matmuls (kT[j] stationary), ONE exp over the
      [128, 2, w] head-pair mega-tile (causally trimmed width w), triangular
      mask on the 128-wide diagonal block only, denominator accumulated in
      bf16 on DVE as independent even/odd chains, attn@V accumulated in PSUM
      (lagging one j so the matmul never waits on exp);
      per head: two accumulated ones-matmuls merge the chains ->
      reciprocal_approx_fast -> gpsimd partition_broadcast -> normalize into
      xT (bf16); hp0's norm is emitted behind hp1's sweep, hp1's is deferred
      into the next chunk's QKV.
  trailing oproj(3), with the final norm broadcast done via a tensor-engine
  matmul (avoids a gpsimd pipeline drain on the critical tail).
Output partials are written as bf16; host upcasts, sums, and adds bo.

Measured on TRN2 (8 cores): 248.7us vs 437.6us for the phase-separated
baseline (tensor-engine active ~227us of a ~209us matmul-issue floor).
"""

import numpy as np
import ml_dtypes

import concourse.bass as bass
import concourse.mybir as mybir
import concourse.tile as tile
from concourse import bacc

BF16 = ml_dtypes.bfloat16
F32 = mybir.dt.float32
BF = mybir.dt.bfloat16

B, S, H = 2, 2048, 2048
NH, KVH, D = 16, 4, 128
G = NH // KVH  # q heads per kv head / per core
N_CORES = 8
SCALE = 1.0 / float(np.sqrt(D))

SQ = 512              # q-chunk width
NQC = S // SQ         # 4 q chunks
NKT = S // 128        # 16 kv tiles / token tiles
NHT = H // 128        # 16 hidden k-tiles
ROWS = G + 2          # 6 projection row-blocks: 4 q heads, k, v
EXPF = mybir.ActivationFunctionType.Exp
IDF = mybir.ActivationFunctionType.Identity


def build_nc(num_devices: int = N_CORES) -> bass.Bass:
    nc = bacc.Bacc("TRN2", num_devices=num_devices)

    # packed layouts: [partition, tile, cols] so one DMA moves many tiles
    hTd = nc.dram_tensor("hTd", [128, NHT, S], BF, kind="ExternalInput").ap()
    wqd = nc.dram_tensor("wqd", [128, NHT, ROWS * 128], BF,
                         kind="ExternalInput").ap()
    bqkv = nc.dram_tensor("bqkv", [128, ROWS], F32, kind="ExternalInput").ap()
    cosT = nc.dram_tensor("cosT", [128, S], BF, kind="ExternalInput").ap()
    sinT = nc.dram_tensor("sinT", [128, S], BF, kind="ExternalInput").ap()
    rotT = nc.dram_tensor("rotT", [128, 128], BF, kind="ExternalInput").ap()
    masks2 = nc.dram_tensor("masks2", [128, 256], BF, kind="ExternalInput").ap()
    wod = nc.dram_tensor("wod", [128, G, H], BF, kind="ExternalInput").ap()
    id128 = nc.dram_tensor("id128", [128, 128], BF, kind="ExternalInput").ap()
    out = nc.dram_tensor("out", [S, H], BF, kind="ExternalOutput").ap()

    with tile.TileContext(nc) as tc:
        with (
            tc.tile_pool(name="consts", bufs=1) as consts,
            tc.tile_pool(name="persist", bufs=1) as persist,
            tc.tile_pool(name="hbuf", bufs=2) as hbuf,
            tc.tile_pool(name="work", bufs=4) as work,
            tc.tile_pool(name="work2", bufs=2) as work2,
            tc.tile_pool(name="obuf", bufs=4) as obuf,
            tc.tile_pool(name="psQ", bufs=2, space="PSUM") as psQ,
            tc.tile_pool(name="psS", bufs=2, space="PSUM") as psS,
            tc.tile_pool(name="psAV", bufs=2, space="PSUM") as psAV,
        ):
            ones_f = consts.tile([128, 1], BF, tag="ones_f", name="ones_f")
            nc.vector.memset(ones_f, 1.0)
            ones_rf = consts.tile([1, 128], F32, tag="ones_rf", name="ones_rf")
            nc.vector.memset(ones_rf, 1.0)
            # preload the exp activation table while DMA streams
            warm_in = consts.tile([128, 1], F32, tag="warm_in", name="warm_in")
            nc.vector.memset(warm_in, 0.0)
            warm_out = consts.tile([128, 1], BF, tag="warm_out", name="warm_out")
            nc.scalar.activation(warm_out, warm_in, EXPF)
            # PE warm-up: dummy matmuls on memset data during the initial DMA
            # wait so HAM unthrottles (1.2 -> 2.4 GHz) before real work
            dummy_w = consts.tile([128, SQ], BF, tag="dummy", name="dummy")
            nc.vector.memset(dummy_w, 0.0)
            for wi in range(10):
                dps = psQ.tile([128, SQ], F32, tag="qkv", name="warmmm")
                nc.tensor.matmul(dps, dummy_w[:, 0:128], dummy_w,
                                 start=True, stop=True)

            # weights + chunk-0 hidden: progressively sized k-tile groups so
            # the first matmuls start after ~330KB while the rest streams
            GRP = [(0, 1), (1, 2), (2, 3), (3, 5), (5, 8), (8, 12), (12, 16)]
            wq_g, h_g = [], []
            h_q = [[None] * 4 for _ in range(NQC)]
            for gi, (lo, hi) in enumerate(GRP):
                wt = persist.tile([128, hi - lo, ROWS * 128], BF,
                                  tag=f"wq{gi}", name=f"wq{gi}")
                nc.sync.dma_start(out=wt, in_=wqd[:, lo:hi, :])
                wq_g.append(wt)
                ht = persist.tile([128, hi - lo, SQ], BF, tag=f"hg{gi}",
                                  name=f"h0_{gi}")
                nc.sync.dma_start(out=ht, in_=hTd[:, lo:hi, 0:SQ])
                h_g.append(ht)
            # small constants: not needed until ~25us in, so their DMA
            # triggers go after the critical weight/hidden ones
            bias_sb = consts.tile([128, ROWS], F32, tag="bias", name="bias")
            nc.sync.dma_start(out=bias_sb, in_=bqkv)
            rt_sb = consts.tile([128, 128], BF, tag="rt", name="rt")
            nc.sync.dma_start(out=rt_sb, in_=rotT)
            mask_sb = consts.tile([128, 2, 128], BF, tag="mask", name="mask")
            nc.sync.dma_start(out=mask_sb, in_=masks2)
            id_sb = consts.tile([128, 128], BF, tag="id", name="id")
            nc.sync.dma_start(out=id_sb, in_=id128)
            cos_sb = persist.tile([128, S], BF, tag="cos", name="cos")
            nc.sync.dma_start(out=cos_sb, in_=cosT)
            sin_sb = persist.tile([128, S], BF, tag="sin", name="sin")
            nc.sync.dma_start(out=sin_sb, in_=sinT)
            wo_sb = persist.tile([128, G, H], BF, tag="wo", name="wo")
            nc.sync.dma_start(out=wo_sb, in_=wod)

            def _grp(kt):
                for gi, (lo, hi) in enumerate(GRP):
                    if kt < hi:
                        return gi, kt - lo
                raise AssertionError

            def wq_ap(kt, m):
                gi, o = _grp(kt)
                return wq_g[gi][:, o, m * 128:(m + 1) * 128]

            def h_ap(c, kt):
                if c == 0:
                    gi, o = _grp(kt)
                    return h_g[gi][:, o, :]
                return h_q[c][kt // 4][:, kt % 4, :]

            # persistent activations (bf16)
            qk_sb = [persist.tile([128, S], BF, tag=f"qk{m}", name=f"qk{m}")
                     for m in range(G + 1)]  # 0..3 q heads, 4 = k
            vT_sb = persist.tile([128, S], BF, tag="vT", name="vT")
            v_sb = [persist.tile([128, 128], BF, tag=f"v{j}", name=f"v{j}")
                    for j in range(NKT)]
            xT_sb = [persist.tile([128, S], BF, tag=f"xT{h}", name=f"xT{h}")
                     for h in range(G)]
            kT = qk_sb[G]

            # rows: m 0..3 -> q head m (RoPE), 4 -> k (RoPE), 5 -> v (plain)
            def row_bias(m, ps, cs):
                """PSUM->SBUF copy with bias; returns rope tmp or None."""
                if m == ROWS - 1:
                    nc.scalar.activation(vT_sb[:, cs], ps, IDF,
                                         bias=bias_sb[:, m:m + 1])
                    return None
                tmp = work.tile([128, SQ], BF, tag="tmp", name="tmp")
                nc.scalar.activation(tmp, ps, IDF, bias=bias_sb[:, m:m + 1])
                return tmp

            def row_rope(m, tmp, cs):
                rp = psAV.tile([128, SQ], F32, tag="av", name="rp")
                nc.tensor.matmul(rp, rt_sb, tmp, start=True, stop=True)
                rot = work.tile([128, SQ], BF, tag="rot", name="rot")
                nc.vector.tensor_mul(rot, rp, sin_sb[:, cs])
                tcos = work.tile([128, SQ], BF, tag="tcos", name="tcos")
                nc.vector.tensor_mul(tcos, tmp, cos_sb[:, cs])
                nc.vector.tensor_add(qk_sb[min(m, G)][:, cs], rot, tcos)

            def oproj_fill_ops(c, fine_dma=False):
                """One micro-op per (token tile, output column block): alloc a
                PSUM tile, 4 accumulated matmuls over heads, copy into the
                per-tile output staging buffer, DMA the row block when done.
                Each closure takes the PSUM pool to use (the one that is free
                during the sweep it is interleaved into)."""
                osbs = {}
                ops = []
                for ti, t in enumerate(range(4 * c, 4 * c + 4)):
                    for n in range(G):
                        def op_fn(pool, t=t, n=n, fine_dma=fine_dma):
                            if t not in osbs:
                                osbs[t] = obuf.tile([128, H], BF, tag="osb",
                                                    name="osb")
                            osb = osbs[t]
                            op = pool.tile([128, SQ], F32,
                                           tag="qkv" if pool is psQ else "av",
                                           name="op")
                            ts_ = slice(t * 128, (t + 1) * 128)
                            for g in range(G):
                                nc.tensor.matmul(
                                    op, xT_sb[g][:, ts_],
                                    wo_sb[:, g, n * SQ:(n + 1) * SQ],
                                    start=(g == 0), stop=(g == G - 1))
                            dst = osb[:, n * SQ:(n + 1) * SQ]
                            # in-sweep copies go to DVE: ACT gates attention
                            # via the exp chain, so keep it off ACT there
                            if fine_dma and n % 2 == 0:
                                nc.scalar.copy(dst, op)
                            else:
                                nc.vector.tensor_copy(dst, op)
                            if fine_dma:
                                nc.sync.dma_start(
                                    out=out[t * 128:(t + 1) * 128,
                                            n * SQ:(n + 1) * SQ],
                                    in_=osb[:, n * SQ:(n + 1) * SQ])
                            elif n == G - 1:
                                nc.sync.dma_start(
                                    out=out[t * 128:(t + 1) * 128, :], in_=osb)
                        ops.append(op_fn)
                return ops

            def attn_chunk(c, fill):
                """Returns the hp1 normalization closure for deferred
                emission (behind the next chunk's first QKV row) so the
                denominator matmuls never stall the tensor queue."""
                cs = slice(c * SQ, (c + 1) * SQ)
                njt = 4 * c + 4
                split = njt >= 8  # even/odd denominator chains (j=0,1 full)
                norms = []
                for hp in range(2):
                    h0, h1 = 2 * hp, 2 * hp + 1
                    if hp == 0:
                        av0 = psAV.tile([128, SQ], F32, tag="av", name="av0")
                        av1 = psAV.tile([128, SQ], F32, tag="av", name="av1")
                    else:
                        av0 = psQ.tile([128, SQ], F32, tag="qkv", name="av0b")
                        av1 = psQ.tile([128, SQ], F32, tag="qkv", name="av1b")
                    daccs = [work2.tile([128, 2, SQ], BF, tag=f"dacc{p}",
                                        name=f"dacc{p}")
                             for p in range(2 if split else 1)]
                    pend = None  # (j, ex, off) awaiting its attn@V matmuls
                    for j in range(njt):
                        i = j - 4 * c
                        off = 128 * i if i > 0 else 0
                        sc = psS.tile([128, 2, SQ], F32, tag="sc", name="sc")
                        for hs, h in ((0, h0), (1, h1)):
                            nc.tensor.matmul(
                                sc[:, hs, off:],
                                kT[:, j * 128:(j + 1) * 128],
                                qk_sb[h][:, c * SQ + off:(c + 1) * SQ],
                                start=True, stop=True,
                            )
                        if pend is not None:
                            pj, pex, poff = pend
                            nc.tensor.matmul(av0[:, poff:], v_sb[pj],
                                             pex[:, 0, poff:],
                                             start=(pj == 0), stop=False)
                            nc.tensor.matmul(av1[:, poff:], v_sb[pj],
                                             pex[:, 1, poff:],
                                             start=(pj == 0), stop=False)
                        remaining = (njt - j) + (njt - 1 if hp == 0 else 0)
                        if j >= 1 and fill and (j % 2 == 1
                                                or len(fill) >= remaining):
                            # psQ is free during hp0 (held by hp1's attn@V
                            # accumulators later); psAV frees once hp0's
                            # normalization has read av0/av1. Spread the ops
                            # evenly so neither sweep runs exp-gated dry.
                            fill.pop(0)(psQ if hp == 0 else psAV)
                        ex = work.tile([128, 2, SQ], BF, tag="ex", name="ex")
                        nc.scalar.activation(ex[:, :, off:], sc[:, :, off:],
                                             EXPF, scale=SCALE)
                        if i >= 0:
                            nc.vector.tensor_mul(ex[:, :, off:off + 128],
                                                 ex[:, :, off:off + 128],
                                                 mask_sb)
                        dacc = daccs[j % 2] if split else daccs[0]
                        if j < (2 if split else 1):
                            nc.vector.tensor_copy(dacc, ex)
                        else:
                            nc.vector.tensor_add(dacc[:, :, off:],
                                                 dacc[:, :, off:],
                                                 ex[:, :, off:])
                        pend = (j, ex, off)
                        if hp == 1 and j == 1:
                            norms[0]()  # hp0 norm behind hp1's first matmuls
                    pj, pex, poff = pend
                    nc.tensor.matmul(av0[:, poff:], v_sb[pj], pex[:, 0, poff:],
                                     start=(pj == 0), stop=True)
                    nc.tensor.matmul(av1[:, poff:], v_sb[pj], pex[:, 1, poff:],
                                     start=(pj == 0), stop=True)

                    def norm(hp=hp, av0=av0, av1=av1, daccs=daccs,
                             mm_bcast=False):
                        if len(daccs) == 2:
                            dm = work2.tile([128, 2, SQ], BF, tag="daccm",
                                            name="daccm")
                            nc.vector.tensor_add(dm, daccs[0], daccs[1])
                        else:
                            dm = daccs[0]
                        for hs, av in ((0, av0), (1, av1)):
                            h = 2 * hp + hs
                            dn = psS.tile([1, SQ], F32, tag="sc", name="dn")
                            nc.tensor.matmul(dn, ones_f, dm[:, hs, :],
                                             start=True, stop=True)
                            rd = work2.tile([1, SQ], F32, tag="rd", name="rd")
                            nc.vector.reciprocal_approx_fast(rd, dn)
                            rdb = work2.tile([128, SQ], F32, tag="rdb",
                                             name="rdb")
                            if mm_bcast:
                                # tensor-engine broadcast: avoids the gpsimd
                                # pipeline drain on the kernel's critical tail
                                bc = psAV.tile([128, SQ], F32, tag="av",
                                               name="bc")
                                nc.tensor.matmul(bc, ones_rf, rd,
                                                 start=True, stop=True)
                                nc.scalar.copy(rdb, bc)
                            else:
                                nc.gpsimd.partition_broadcast(rdb, rd)
                            nc.vector.tensor_mul(xT_sb[h][:, cs], av, rdb)
                    norms.append(norm)
                return norms[1]

            ROW_ORDER = (G, ROWS - 1, 0, 1, 2, 3)  # k, v, q0..q3
            pending_norm = None
            fill = []  # oproj micro-ops carried across chunks as filler
            for c in range(NQC):
                cs = slice(c * SQ, (c + 1) * SQ)
                # prefetch next chunk's hidden tiles
                if c + 1 < NQC:
                    for q in range(4):
                        ht = hbuf.tile([128, 4, SQ], BF, tag=f"h{q}",
                                       name=f"h{c + 1}_{q}")
                        nc.sync.dma_start(
                            out=ht,
                            in_=hTd[:, 4 * q:4 * q + 4,
                                    (c + 1) * SQ:(c + 2) * SQ])
                        h_q[c + 1][q] = ht
                # ---- QKV projection + RoPE ----
                if c == 0:
                    # k-tile-outer so compute starts as DMA streams in;
                    # 6 concurrent accumulators spread over all three pools
                    pools = {0: psQ, 1: psQ, 2: psS, 3: psS, 4: psAV, 5: psAV}
                    tags = {0: "qkv", 1: "qkv", 2: "sc", 3: "sc",
                            4: "av", 5: "av"}
                    accs = {m: pools[m].tile([128, SQ], F32, tag=tags[m],
                                             name=f"acc{m}")
                            for m in range(ROWS)}
                    for kt in range(NHT):
                        for m in range(ROWS):
                            nc.tensor.matmul(
                                accs[m], wq_ap(kt, m), h_ap(0, kt),
                                start=(kt == 0), stop=(kt == NHT - 1),
                            )
                    tmps = {m: row_bias(m, accs[m], cs) for m in ROW_ORDER}
                    for m in ROW_ORDER:
                        if tmps[m] is not None:
                            row_rope(m, tmps[m], cs)
                else:
                    prev = None  # stagger rope behind next row's matmuls
                    for m in ROW_ORDER:
                        # k and v rows accumulate in the attn@V pool, which
                        # is free at the chunk boundary; q rows wait for the
                        # deferred norm to release the psQ slots
                        pool, tag = (psAV, "av") if m >= G else (psQ, "qkv")
                        ps = pool.tile([128, SQ], F32, tag=tag, name="mm")
                        for kt in range(NHT):
                            nc.tensor.matmul(
                                ps, wq_ap(kt, m), h_ap(c, kt),
                                start=(kt == 0), stop=(kt == NHT - 1),
                            )
                        if pending_norm is not None:
                            # prev chunk's hp1 norm: its denominator matmuls
                            # land behind this k-row so they never wait
                            pending_norm()
                            pending_norm = None
                        if prev is not None:
                            row_rope(prev[0], prev[1], cs)
                            prev = None
                        tmp = row_bias(m, ps, cs)
                        if tmp is not None:
                            prev = (m, tmp)
                    if prev is not None:
                        row_rope(prev[0], prev[1], cs)
                # ---- transpose this chunk's v tiles ----
                for j in range(4 * c, 4 * c + 4):
                    tp = psS.tile([128, 128], BF, tag="sc", name="tp")
                    nc.tensor.transpose(tp, vT_sb[:, j * 128:(j + 1) * 128],
                                        id_sb)
                    nc.scalar.copy(v_sb[j], tp)
                # ---- attention with oproj(c-1) interleaved ----
                if c > 0:
                    fill.extend(oproj_fill_ops(c - 1))
                pending_norm = attn_chunk(c, fill)
            pending_norm(mm_bcast=True)
            for fi, fn in enumerate(oproj_fill_ops(NQC - 1, fine_dma=True)):
                fn(psQ if fi % 2 else psAV)
    nc.compile()
    return nc


def make_in_maps(hidden_states, cos, sin, Wq, bq, Wk, bk, Wv, bv, Wo, bo):
    """Host-side shard/pack. Returns list of 8 input dicts."""
    f32 = np.float32
    cosT = np.ascontiguousarray(np.asarray(cos).T).astype(BF16)
    sinT = np.ascontiguousarray(np.asarray(sin).T).astype(BF16)
    R = np.zeros((128, 128), f32)
    for d in range(64):
        R[d, d + 64] = -1.0
        R[d + 64, d] = 1.0
    rotT = np.ascontiguousarray(R.T).astype(BF16)
    # triangular mask for the diagonal 128-block, duplicated per head-pair
    p = np.arange(128)[:, None]
    q = np.arange(128)[None, :]
    tri = (q >= p).astype(BF16)
    masks2 = np.concatenate([tri, tri], axis=1)  # [128, 256]
    id128 = np.eye(128, dtype=BF16)

    in_maps = []
    for core in range(N_CORES):
        b, k = core // 4, core % 4
        hT = np.ascontiguousarray(np.asarray(hidden_states[b]).T)  # [H, S]
        hTd = np.ascontiguousarray(
            hT.reshape(NHT, 128, S).transpose(1, 0, 2)).astype(BF16)
        wq = Wq[512 * k:512 * (k + 1)]            # [512, H]
        wk = Wk[128 * k:128 * (k + 1)]            # [128, H]
        wv = Wv[128 * k:128 * (k + 1)]
        wqkvT = np.ascontiguousarray(
            np.concatenate([wq, wk, wv], axis=0).T)  # [H, 768]
        wqd = np.ascontiguousarray(
            wqkvT.reshape(NHT, 128, ROWS * 128).transpose(1, 0, 2)
        ).astype(BF16)                             # [128, 16, 768]
        bqkv = np.concatenate(
            [bq[512 * k:512 * (k + 1)], bk[128 * k:128 * (k + 1)],
             bv[128 * k:128 * (k + 1)]]
        ).astype(f32).reshape(ROWS, 128).T.copy()  # [128, ROWS]
        woT = np.ascontiguousarray(Wo[:, 512 * k:512 * (k + 1)].T)  # [512, H]
        wod = np.ascontiguousarray(
            woT.reshape(G, 128, H).transpose(1, 0, 2)).astype(BF16)
        in_maps.append({
            "hTd": hTd, "wqd": wqd, "bqkv": bqkv,
            "cosT": cosT, "sinT": sinT, "masks2": masks2, "rotT": rotT,
            "wod": wod, "id128": id128,
        })
    return in_maps


_NC = None


def kernel(**inputs) -> np.ndarray:
    global _NC
    from concourse.bass_utils import run_bass_kernel_spmd

    if _NC is None:
        _NC = build_nc()
    in_maps = make_in_maps(**inputs)
    res = run_bass_kernel_spmd(_NC, in_maps, core_ids=list(range(N_CORES)))
    out = np.zeros((B, S, H), np.float32)
    for core in range(N_CORES):
        out[core // 4] += np.asarray(res.results[core]["out"], np.float32)
    out += np.asarray(inputs["bo"], np.float32)
    return out
